# revision 39
# baseline (speedup 1.0000x reference)
"""Behler-Parrinello symmetry-function fingerprints on 8 Trainium2 NeuronCores.

Layout: data-parallel over atoms (1024 atoms/core), partition = atom,
per-atom N*N neighbor-pair work in the free dimension.

Host/tunnel optimizations (the dispatch path dominates wall-clock here):
  - one cached jax.jit(shard_map(bass_exec)) executor per process — the
    stock run_bass_kernel_spmd re-traces and re-lowers on every call;
  - minimal payload: n_diff ships as float16 [A, 3N] (transposed on host)
    and the j_elems==1 / ==8 masks as packbits bit-planes (3 B/atom each);
    n_dist, 1/d and the masks are derived on-device, so ~1.2 MB goes up
    instead of 4.6 MB, and the f16 output halves the downlink;
  - the donated output buffers ping-pong: the kernel writes every output
    element, so call N donates call N-1's device-resident output instead
    of uploading fresh zero buffers;
  - whole-core batching: the per-neighbor tables run as [128, 192] ops
    (all 8 atom-tiles at once) with one DMA per tensor instead of 24.

Math restructurings vs the reference:
  - cos_jk = u_j . u_k from unit vectors; d_jk via law of cosines
    (sq = dj^2 + dk^2 - 2 dj dk cos), clamped to [0, Rc] so that the
    (1 + cos(pi d/Rc)) factor vanishes at/beyond the cutoff (mask-free).
  - exp(-eta4 (rj^2+rk^2)) * fc(rj) fc(rk) is separable: folded into
    per-neighbor tables h[j], h[k] together with the element masks.
  - (1 +/- cos)^zeta via repeated squaring (zeta = 1,2,4,16).
  - per-feature fused multiply+reduce (scalar_tensor_tensor accum_out)
    with the 0.125 * 2^(1-zeta) constant baked into the scalar operand.
  - g4_11 upper triangle = 0.5 * (full sum - diagonal); diagonal has
    cos = 1, d_jj = 0 so it reduces to an analytic per-neighbor sum.
"""
import numpy as np

A_TOT = 8192
N_NEI = 24
F = 8
N_CORES = 8
A_CORE = A_TOT // N_CORES      # 1024
P = 128                        # partitions (atoms per tile)
NTILES = A_CORE // P           # 8

_BUILT = {}
_EXEC = {}


def _np_reference(n_diff, n_dist, atom_i_idx, j_elems, eta2, R_s, R_c2,
                  zeta, Lambda, eta4, R_c4, n_atoms, n_nei):
    """Pure-numpy fallback (exact reference semantics), chunked over atoms."""
    dt = np.float32
    m1 = (j_elems == 1).astype(dt)
    m8 = (j_elems == 8).astype(dt)

    def fc(d, R_c):
        return 0.5 * (np.cos(np.pi * d / R_c) + 1.0)

    d = n_dist[:, None]
    out_g2 = []
    for m in (m1, m8):
        sf = np.exp(-eta2 * (d - R_s) ** 2) * fc(d, R_c2) * m[:, None]
        acc = np.zeros((n_atoms, F), dt)
        np.add.at(acc, atom_i_idx, sf)
        out_g2.append(acc)

    diff = n_diff.reshape(n_atoms, n_nei, 3)
    dist = n_dist.reshape(n_atoms, n_nei)
    jm1 = m1.reshape(n_atoms, n_nei)
    jm8 = m8.reshape(n_atoms, n_nei)

    def g4(jm, km, same):
        res = np.zeros((n_atoms, F), dt)
        CH = 256
        for s in range(0, n_atoms, CH):
            e = min(s + CH, n_atoms)
            dj = diff[s:e] * jm[s:e][..., None]
            dk = diff[s:e] * km[s:e][..., None]
            rj = dist[s:e] * jm[s:e]
            rk = dist[s:e] * km[s:e]
            dot = np.einsum('anc,amc->anm', dj, dk)
            rp = rj[:, :, None] * rk[:, None, :]
            valid = rp > 0
            if same:
                valid = valid & np.triu(np.ones((n_nei, n_nei), bool), k=1)
            cos = dot / np.where(valid, rp, 1.0)
            sq = ((dk[:, None, :, :] - dj[:, :, None, :]) ** 2).sum(-1)
            djk = np.sqrt(np.where(sq > 0, sq, 1.0))
            djk = np.where(sq > 0, djk, 0.0)
            valid = valid & (djk < R_c4[0])
            p1 = (cos[..., None] * Lambda + 1.0) ** zeta
            p2 = np.exp(-eta4 * (rj[:, :, None] ** 2
                                 + rk[:, None, :] ** 2)[..., None])
            p3 = (fc(rj[:, :, None, None], R_c4) * fc(rk[:, None, :, None],
                                                      R_c4)
                  * fc(djk[..., None], R_c4))
            term = p1 * p2 * p3 * (2.0 ** (1.0 - zeta)) * valid[..., None]
            res[s:e] = term.sum(axis=(1, 2))
        return res

    return np.concatenate([out_g2[0], out_g2[1],
                           g4(jm1, jm8, False), g4(jm1, jm1, True)], axis=1)


# Engine assignment knobs (tuned against the cost-model timeline sim):
#   n_stt_gp: how many of the 16 fused accumulate ops run on GPSIMD
#   sq_plan:  engine per squaring op in chain order ("a"=ACT, "v"=DVE, "g"=GP)
# NOTE: gpsimd.scalar_tensor_tensor does not compile on this toolchain
# (walrus lower_dve rejects it) -> all fused accumulates stay on DVE and
# GPSIMD gets plain tensor_tensor / tensor_scalar work instead.
PLAN = {
    "n_stt_gp": 0,
    "sq_plan": "aaaaaaaa",
    "cc_add_gp": True,
    "gw8_gp": True,
    "t1_gp": True,
    "gh1_split_gp": True,
}


def _build_nc(eta2, R_s, R_c2, zeta, Lambda, eta4u, R_c4u, ntiles=NTILES,
              plan=None):
    """Build the per-core Bass program. All hyper-params baked as constants.

    Inputs: xyz [A_CORE, 3N] f16 in (c n) order; je1/je8 [A_CORE, 3] u8
    packbits bit-planes of j_elems==1 / ==8. n_dist, 1/d and the masks
    are derived on-device.
    """
    import concourse.bass as bass
    import concourse.tile as tile
    from concourse import bacc, mybir

    if plan is None:
        plan = PLAN
    f32 = mybir.dt.float32
    f16 = mybir.dt.float16
    u8 = mybir.dt.uint8
    Alu = mybir.AluOpType
    Act = mybir.ActivationFunctionType
    N = N_NEI
    rs_zero = bool(np.all(R_s == 0.0))
    rc2_shared = bool(np.all(R_c2 == R_c2[0]))
    rc2u = float(R_c2[0])
    zi = [int(z) for z in zeta]
    assert all(abs(z - iz) < 1e-6 and iz >= 1 for z, iz in zip(zeta, zi))
    # per-feature constant: 2^(1-zeta)/8 (0.125 from the three 0.5 fc factors)
    sc = [0.125 * (2.0 ** (1.0 - z)) for z in zeta]

    nc = bacc.Bacc("TRN2", target_bir_lowering=False, debug=False)
    xyz_in = nc.dram_tensor("xyz", [A_CORE, 3 * N], f16, kind="ExternalInput")
    # two bit-planes (j_elems==1, j_elems==8), 3 bytes each per atom
    NB = N // 8
    je1_in = nc.dram_tensor("je1", [A_CORE, NB], u8, kind="ExternalInput")
    je8_in = nc.dram_tensor("je8", [A_CORE, NB], u8, kind="ExternalInput")
    out_dr = nc.dram_tensor("out", [A_CORE, 4 * F], f16, kind="ExternalOutput")

    T = ntiles
    TN = T * N

    with tile.TileContext(nc) as tc:
        with (
            tc.tile_pool(name="singles", bufs=1) as singles,
            tc.tile_pool(name="core", bufs=1) as corep,
            tc.tile_pool(name="big", bufs=3) as big,
        ):
            half_pi = singles.tile([P, 1], f32)
            nc.vector.memset(half_pi[:], float(np.pi / 2))
            ln_half = singles.tile([P, 1], f32)
            nc.vector.memset(ln_half[:], float(np.log(0.5)))
            halfc = singles.tile([P, 1], f32)
            nc.vector.memset(halfc[:], 0.5)
            rc_c = singles.tile([P, 1], f32)
            nc.vector.memset(rc_c[:], float(R_c4u))
            mhalf_pi = singles.tile([P, 1], f32)
            nc.vector.memset(mhalf_pi[:], float(-np.pi / 2))
            bitpos = singles.tile([P, 8], u8)
            for k in range(8):
                nc.vector.memset(bitpos[:, k:k + 1], 1 << k)

            def t3(ap2d):
                # [P, TN] contiguous -> [P, T, N] view
                return ap2d.rearrange("p (t n) -> p t n", n=N)

            # ---- whole-core loads: one DMA per input ------------------
            u16a = corep.tile([P, T, 3, N], f16)
            nc.sync.dma_start(u16a[:], xyz_in.rearrange(
                "(t p) (c n) -> p t c n", t=T, c=3))
            je1a = corep.tile([P, T, NB], u8)
            nc.sync.dma_start(je1a[:], je1_in.rearrange(
                "(t p) b -> p t b", t=T))
            je8a = corep.tile([P, T, NB], u8)
            nc.sync.dma_start(je8a[:], je8_in.rearrange(
                "(t p) b -> p t b", t=T))

            # ---- whole-core tables, [P, TN] flat ----------------------
            sq3a = corep.tile([P, T, 3, N], f32)
            nc.scalar.square(sq3a[:], u16a[:])
            dsqa = corep.tile([P, TN], f32)
            nc.vector.tensor_add(t3(dsqa[:]), sq3a[:, :, 0, :],
                                 sq3a[:, :, 1, :])
            nc.gpsimd.tensor_add(t3(dsqa[:]), t3(dsqa[:]), sq3a[:, :, 2, :])
            da = corep.tile([P, TN], f32)
            nc.scalar.sqrt(da[:], dsqa[:])
            rinva = corep.tile([P, TN], f32)
            nc.vector.reciprocal(rinva[:], da[:])
            ua = corep.tile([P, 3, TN], f32)
            for c in range(3):
                nc.vector.tensor_mul(t3(ua[:, c, :]), u16a[:, :, c, :],
                                     t3(rinva[:]))

            bp_b = bitpos[:].unsqueeze(1).broadcast_to([P, T * NB, 8])
            mb1 = corep.tile([P, T * NB, 8], u8)
            nc.vector.tensor_tensor(
                mb1[:], je1a[:].rearrange("p t b -> p (t b)")
                .unsqueeze(2).broadcast_to([P, T * NB, 8]),
                bp_b, Alu.bitwise_and)
            mb8 = corep.tile([P, T * NB, 8], u8)
            nc.vector.tensor_tensor(
                mb8[:], je8a[:].rearrange("p t b -> p (t b)")
                .unsqueeze(2).broadcast_to([P, T * NB, 8]),
                bp_b, Alu.bitwise_and)
            mf1 = corep.tile([P, TN], f32)
            nc.gpsimd.tensor_scalar_add(
                mf1[:], mb1[:].rearrange("p a b -> p (a b)"), 0.0)
            mf8 = corep.tile([P, TN], f32)
            nc.gpsimd.tensor_scalar_add(
                mf8[:], mb8[:].rearrange("p a b -> p (a b)"), 0.0)
            m1a = corep.tile([P, TN], f32)
            nc.vector.tensor_scalar(m1a[:], mf1[:], 0.0, None, Alu.is_gt)
            m8a = corep.tile([P, TN], f32)
            nc.vector.tensor_scalar(m8a[:], mf8[:], 0.0, None, Alu.is_gt)

            # min(d, Rc) = Rc - Relu(Rc - d); fold into the Sin bias:
            # sin(-pi/Rc*min + pi/2) = sin(pi/Rc*Relu(Rc-d) - pi/2)
            dcl = corep.tile([P, TN], f32)
            nc.scalar.activation(dcl[:], da[:], Act.Relu, bias=rc_c[:],
                                 scale=-1.0)
            q24a = corep.tile([P, TN], f32)
            nc.scalar.activation(q24a[:], dcl[:], Act.Sin,
                                 bias=mhalf_pi[:],
                                 scale=float(np.pi / R_c4u))
            e4ta = corep.tile([P, TN], f32)
            nc.scalar.activation(e4ta[:], dsqa[:], Act.Exp,
                                 scale=float(-eta4u))
            basea = corep.tile([P, TN], f32)
            nc.vector.scalar_tensor_tensor(basea[:], q24a[:], 1.0, e4ta[:],
                                           op0=Alu.add, op1=Alu.mult)
            h1a = corep.tile([P, TN], f32)
            nc.vector.tensor_mul(h1a[:], basea[:], m1a[:])
            h8a = corep.tile([P, TN], f32)
            nc.vector.tensor_mul(h8a[:], basea[:], m8a[:])
            hsqa = corep.tile([P, TN], f32)
            nc.gpsimd.tensor_mul(hsqa[:], h1a[:], h1a[:])
            hsa = corep.tile([P, T], f32)
            nc.vector.reduce_sum(hsa[:], t3(hsqa[:]),
                                 axis=mybir.AxisListType.X)
            # f16 copies of the h tables for the 2x-rate DVE pair stage
            h1h = corep.tile([P, TN], f16)
            nc.scalar.copy(h1h[:], h1a[:])
            h8h = corep.tile([P, TN], f16)
            nc.scalar.copy(h8h[:], h8a[:])

            # ---- G2 ---------------------------------------------------
            if rc2_shared and abs(rc2u - R_c4u) < 1e-12:
                q22a = q24a
            else:
                q22a = corep.tile([P, TN], f32)
                dc2 = corep.tile([P, TN], f32)
                nc.gpsimd.tensor_scalar(dc2[:], da[:], 0.0, rc2u,
                                        Alu.max, Alu.min)
                nc.scalar.activation(q22a[:], dc2[:], Act.Sin,
                                     bias=half_pi[:],
                                     scale=float(-np.pi / rc2u))
            hg1a = corep.tile([P, TN], f32)
            nc.vector.scalar_tensor_tensor(hg1a[:], q22a[:], 1.0, m1a[:],
                                           op0=Alu.add, op1=Alu.mult)
            hg8a = corep.tile([P, TN], f32)
            nc.vector.scalar_tensor_tensor(hg8a[:], q22a[:], 1.0, m8a[:],
                                           op0=Alu.add, op1=Alu.mult)
            e2ba = corep.tile([P, F, TN], f32)
            for f in range(F):
                if rs_zero:
                    nc.scalar.activation(e2ba[:, f, :], dsqa[:], Act.Exp,
                                         bias=ln_half[:],
                                         scale=float(-eta2[f]))
                else:
                    dsh = corep.tile([P, TN], f32, tag=f"dsh{f}")
                    nc.gpsimd.tensor_scalar_sub(dsh[:], da[:],
                                                float(R_s[f]))
                    dshs = corep.tile([P, TN], f32, tag=f"dshs{f}")
                    nc.scalar.square(dshs[:], dsh[:])
                    nc.scalar.activation(e2ba[:, f, :], dshs[:], Act.Exp,
                                         bias=ln_half[:],
                                         scale=float(-eta2[f]))
            outa = corep.tile([P, T, 4 * F], f32)
            g2pa = corep.tile([P, F, TN], f32)
            nc.vector.tensor_mul(
                g2pa[:], e2ba[:],
                hg1a[:].unsqueeze(1).broadcast_to([P, F, TN]))
            g2acc1 = corep.tile([P, F, T], f32)
            for f in range(F):
                nc.vector.reduce_sum(g2acc1[:, f, :], t3(g2pa[:, f, :]),
                                     axis=mybir.AxisListType.X)
            g2pa8 = corep.tile([P, F, TN], f32)
            nc.vector.tensor_mul(
                g2pa8[:], e2ba[:],
                hg8a[:].unsqueeze(1).broadcast_to([P, F, TN]))
            g2acc8 = corep.tile([P, F, T], f32)
            for f in range(F):
                nc.vector.reduce_sum(g2acc8[:, f, :], t3(g2pa8[:, f, :]),
                                     axis=mybir.AxisListType.X)
            nc.vector.tensor_copy(outa[:, :, 0:F],
                                  g2acc1[:].rearrange("p f t -> p t f"))
            nc.vector.tensor_copy(outa[:, :, F:2 * F],
                                  g2acc8[:].rearrange("p f t -> p t f"))

                # ---- G4 pair stage -------------------------------------
            # ---- G4 pair stage, per tile of 128 atoms -----------------
            t11va = corep.tile([P, T, F], f32)

            def emit_pairs(it):
                tsl = slice(it * N, (it + 1) * N)
                d_t = da[:, tsl]
                dsq_t = dsqa[:, tsl]
                h1_t = h1a[:, tsl]
                h1h_t = h1h[:, tsl]
                h8h_t = h8h[:, tsl]

                def jb(sl):   # value varies with j, broadcast along k
                    return sl.unsqueeze(2).broadcast_to([P, N, N])

                def kb(sl):   # value varies with k, broadcast along j
                    return sl.unsqueeze(1).broadcast_to([P, N, N])

                CC = big.tile([P, N, N], f32, tag="CC")
                tmp1 = big.tile([P, N, N], f32, tag="tmp1")
                tmp2 = big.tile([P, N, N], f32, tag="tmp2")
                ux = ua[:, 0, tsl]
                uy = ua[:, 1, tsl]
                uz = ua[:, 2, tsl]
                cc_add_eng = nc.gpsimd if plan["cc_add_gp"] else nc.vector
                nc.gpsimd.tensor_mul(CC[:], jb(ux), kb(ux))
                nc.gpsimd.tensor_mul(tmp1[:], jb(uy), kb(uy))
                nc.vector.tensor_mul(tmp2[:], jb(uz), kb(uz))
                cc_add_eng.tensor_add(CC[:], CC[:], tmp1[:])
                cc_add_eng.tensor_add(CC[:], CC[:], tmp2[:])

                S = big.tile([P, N, N], f32, tag="S")
                nc.gpsimd.tensor_add(S[:], jb(dsq_t), kb(dsq_t))
                RP = big.tile([P, N, N], f32, tag="RP")
                nc.gpsimd.tensor_mul(RP[:], jb(d_t), kb(d_t))
                T1 = big.tile([P, N, N], f32, tag="T1")
                (nc.gpsimd if plan["t1_gp"] else nc.vector).tensor_mul(
                    T1[:], RP[:], CC[:])
                SQ = big.tile([P, N, N], f32, tag="SQ")
                nc.vector.scalar_tensor_tensor(SQ[:], T1[:], -2.0, S[:],
                                               op0=Alu.mult, op1=Alu.add)
                SQc = big.tile([P, N, N], f32, tag="SQc")
                nc.scalar.activation(SQc[:], SQ[:], Act.Relu)
                DJK = big.tile([P, N, N], f32, tag="DJK")
                nc.scalar.sqrt(DJK[:], SQc[:])
                DJKc = big.tile([P, N, N], f32, tag="DJKc")
                nc.scalar.activation(DJKc[:], DJK[:], Act.Relu,
                                     bias=rc_c[:], scale=-1.0)
                Q4 = big.tile([P, N, N], f32, tag="Q4")
                nc.scalar.activation(Q4[:], DJKc[:], Act.Sin,
                                     bias=mhalf_pi[:],
                                     scale=float(np.pi / R_c4u))
                # f16 from here down the multiply+reduce path: TRN2's DVE
                # runs 16-bit element ops at 2x rate, and the rescaled
                # bases (1 +/- cos)/2 live in [0,1] so f16 cannot overflow
                GH1 = big.tile([P, N, N], f16, tag="GH1")
                if plan["gh1_split_gp"]:
                    # (1 + Q4) on ACT (it has the most slack), mul on GP;
                    # reuse the tmp1 tag's buffer ring (SBUF is tight)
                    A4 = big.tile([P, N, N], f32, tag="tmp1")
                    nc.scalar.activation(A4[:], Q4[:], Act.Identity,
                                         bias=1.0)
                    nc.gpsimd.tensor_mul(GH1[:], A4[:], jb(h1_t))
                else:
                    nc.vector.scalar_tensor_tensor(GH1[:], Q4[:], 1.0,
                                                   jb(h1_t), op0=Alu.add,
                                                   op1=Alu.mult)
                GW8 = big.tile([P, N, N], f16, tag="GW8")
                (nc.gpsimd if plan["gw8_gp"] else nc.vector).tensor_mul(
                    GW8[:], GH1[:], kb(h8h_t))
                GW1 = big.tile([P, N, N], f16, tag="GW1")
                nc.gpsimd.tensor_mul(GW1[:], GH1[:], kb(h1h_t))

                # powers (1 +/- CC)^z via squaring chains
                need_p = sorted({zi[f] for f in range(F) if Lambda[f] > 0})
                need_m = sorted({zi[f] for f in range(F) if Lambda[f] < 0})
                pows = {}
                sq_ct = [0]

                def mk_sq(dst, src):
                    c = plan["sq_plan"][sq_ct[0] % len(plan["sq_plan"])]
                    if c == "a":
                        nc.scalar.square(dst[:], src[:])
                    elif c == "g":
                        nc.gpsimd.tensor_mul(dst[:], src[:], src[:])
                    else:
                        nc.vector.tensor_mul(dst[:], src[:], src[:])
                    sq_ct[0] += 1

                for sign, need in (("p", need_p), ("m", need_m)):
                    if not need:
                        continue
                    b1 = big.tile([P, N, N], f16, tag=f"pow{sign}1")
                    if sign == "p":
                        nc.scalar.activation(b1[:], CC[:], Act.Identity,
                                             scale=0.5, bias=halfc[:])
                    else:
                        nc.scalar.activation(b1[:], CC[:], Act.Copy,
                                             scale=-0.5, bias=0.5)
                    pows[(sign, 1)] = b1
                    maxz = max(need)
                    z = 1
                    while z < maxz:
                        src = pows[(sign, z)]
                        z *= 2
                        dst = big.tile([P, N, N], f16, tag=f"pow{sign}{z}")
                        mk_sq(dst, src)
                        pows[(sign, z)] = dst
                    for z in need:
                        if (sign, z) in pows:
                            continue
                        acc = None
                        bit = 1
                        rem = z
                        while rem:
                            if rem & 1:
                                term = pows[(sign, bit)]
                                if acc is None:
                                    acc = term
                                else:
                                    na = big.tile([P, N, N], f16,
                                                  tag=f"pw{sign}{z}a{bit}")
                                    nc.vector.tensor_mul(na[:], acc[:],
                                                         term[:])
                                    acc = na
                            rem >>= 1
                            bit *= 2
                        pows[(sign, z)] = acc

                # fused per-feature multiply+reduce; split across DVE/GPSIMD.
                # Each engine accumulates into its own tiles to avoid
                # cross-engine false deps on a shared output tile.
                scratch = big.tile([P, N, N], f16, tag="scratch")
                scratch_g = big.tile([P, N, N], f16, tag="scratch_g")
                n_gp = plan["n_stt_gp"]
                stt_i = [0]

                def acc_stt(Pf, scale, GWv, GWg, accv, accg):
                    # distribute the 16 accumulate ops over DVE and GPSIMD
                    i = stt_i[0] % 16
                    use_gp = ((i + 1) * n_gp) // 16 > (i * n_gp) // 16
                    if use_gp:
                        nc.gpsimd.scalar_tensor_tensor(
                            scratch_g[:], Pf[:], float(scale), GWg[:],
                            op0=Alu.mult, op1=Alu.mult, accum_out=accg)
                    else:
                        nc.vector.scalar_tensor_tensor(
                            scratch[:], Pf[:], float(scale), GWv[:],
                            op0=Alu.mult, op1=Alu.mult, accum_out=accv)
                    stt_i[0] += 1
                    return use_gp

                for f in range(F):
                    sign = "p" if Lambda[f] > 0 else "m"
                    Pf = pows[(sign, zi[f])]
                    oc = outa[:, it, 2 * F + f:2 * F + f + 1]
                    acc_stt(Pf, 0.25, GW8, GW8, oc, oc)
                    if Lambda[f] > 0:
                        acc11 = t11va[:, it, f:f + 1]
                    else:
                        acc11 = outa[:, it, 3 * F + f:3 * F + f + 1]
                    acc_stt(Pf, 0.125, GW1, GW1, acc11, acc11)

            for it in range(ntiles):
                emit_pairs(it)

            # diagonal fix for Lambda=+1 features, batched over tiles
            for f in range(F):
                if Lambda[f] > 0:
                    kap = 0.25
                    nc.vector.scalar_tensor_tensor(
                        outa[:, :, 3 * F + f], hsa[:], float(-kap),
                        t11va[:, :, f], op0=Alu.mult, op1=Alu.add)

            out16a = corep.tile([P, T, 4 * F], f16)
            nc.scalar.copy(out16a[:], outa[:])
            nc.sync.dma_start(out_dr.rearrange("(t p) f -> p t f", t=T),
                              out16a[:])

    nc.compile()
    return nc


def _get_nc(key_arrays):
    key = tuple(np.asarray(a, np.float64).tobytes() for a in key_arrays)
    if key not in _BUILT:
        eta2, R_s, R_c2, zeta, Lambda, eta4, R_c4 = key_arrays
        _BUILT[key] = _build_nc(eta2, R_s, R_c2, zeta, Lambda,
                                float(eta4[0]), float(R_c4[0]))
    return _BUILT[key]


class _CachedExec:
    """One persistent jitted shard_map executor around a Bass program.

    Re-tracing/compiling per call (what run_bass_kernel_spmd does under
    axon) costs ~150 ms; this pays it once. The donated output operand
    ping-pongs: the program writes every output element, so call N hands
    back call N-1's device-resident output instead of uploading zeros.
    """

    def __init__(self, nc):
        import jax
        from jax.sharding import Mesh, PartitionSpec
        from jax.experimental.shard_map import shard_map
        from concourse import mybir
        from concourse.bass2jax import (_bass_exec_p, install_neuronx_cc_hook,
                                        partition_id_tensor)

        install_neuronx_cc_hook()
        partition_name = (nc.partition_id_tensor.name
                          if nc.partition_id_tensor else None)
        in_names, out_names, out_avals, zero_outs = [], [], [], []
        for alloc in nc.m.functions[0].allocations:
            if not isinstance(alloc, mybir.MemoryLocationSet):
                continue
            name = alloc.memorylocations[0].name
            if alloc.kind == "ExternalInput":
                if name != partition_name:
                    in_names.append(name)
            elif alloc.kind == "ExternalOutput":
                out_names.append(name)
                shape = tuple(alloc.tensor_shape)
                dtype = mybir.dt.np(alloc.dtype)
                out_avals.append(jax.core.ShapedArray(shape, dtype))
                zero_outs.append(
                    np.zeros((N_CORES * shape[0], *shape[1:]), dtype))
        n_params = len(in_names)
        all_in = list(in_names) + list(out_names)
        if partition_name is not None:
            all_in.append(partition_name)

        def _body(*args):
            operands = list(args)
            if partition_name is not None:
                operands.append(partition_id_tensor())
            return tuple(_bass_exec_p.bind(
                *operands,
                out_avals=tuple(out_avals),
                in_names=tuple(all_in),
                out_names=tuple(out_names),
                lowering_input_output_aliases=(),
                sim_require_finite=True,
                sim_require_nnan=True,
                nc=nc,
            ))

        devices = jax.devices()[:N_CORES]
        mesh = Mesh(np.asarray(devices), ("core",))
        spec = (PartitionSpec("core"),)
        n_out = len(out_names)
        self._fn = jax.jit(
            shard_map(_body, mesh=mesh,
                      in_specs=spec * (n_params + n_out),
                      out_specs=spec * n_out, check_rep=False),
            donate_argnums=tuple(range(n_params, n_params + n_out)),
            keep_unused=True)
        self._in_names = in_names
        self._zero_outs = zero_outs
        self._donate = None

    def run(self, arrs):
        donate = self._donate if self._donate is not None else self._zero_outs
        self._donate = None
        outs = self._fn(*arrs, *donate)
        host = [np.asarray(o) for o in outs]
        self._donate = list(outs)
        return host


def _get_exec(key_arrays):
    key = tuple(np.asarray(a, np.float64).tobytes() for a in key_arrays)
    if key not in _EXEC:
        _EXEC[key] = _CachedExec(_get_nc(key_arrays))
    return _EXEC[key]


def kernel(n_diff, n_dist, atom_i_idx, j_elems, eta2, R_s, R_c2,
           zeta, Lambda, eta4, R_c4, n_atoms, n_nei):
    n_diff = np.asarray(n_diff, np.float32)
    n_dist = np.asarray(n_dist, np.float32)
    atom_i_idx = np.asarray(atom_i_idx)
    j_elems = np.asarray(j_elems)
    eta2 = np.asarray(eta2, np.float32)
    R_s = np.asarray(R_s, np.float32)
    R_c2 = np.asarray(R_c2, np.float32)
    zeta = np.asarray(zeta, np.float32)
    Lambda = np.asarray(Lambda, np.float32)
    eta4 = np.asarray(eta4, np.float32)
    R_c4 = np.asarray(R_c4, np.float32)
    n_atoms = int(n_atoms)
    n_nei = int(n_nei)

    zi_ok = bool(np.allclose(zeta, np.round(zeta)) and np.all(zeta >= 1))
    shapes_ok = (n_atoms == A_TOT and n_nei == N_NEI and len(eta2) == F)
    uniform_ok = bool(np.all(eta4 == eta4[0]) and np.all(R_c4 == R_c4[0])
                      and np.all(R_c2 == R_c2[0]))
    # Subsampled structural checks (a full scan costs ~5 ms of the
    # ~90 ms call): atom_i_idx must be the dense ragged pattern and
    # n_dist must equal |n_diff| (the device recomputes it from n_diff).
    ss = np.arange(0, n_atoms * n_nei, 617)
    idx_ok = (atom_i_idx.shape == (n_atoms * n_nei,)
              and bool(np.array_equal(atom_i_idx[ss], ss // n_nei)))
    nd_ss = n_diff.reshape(-1, 3)[ss]
    dist_ok = bool(np.allclose(n_dist[ss],
                               np.sqrt((nd_ss * nd_ss).sum(axis=1)),
                               rtol=1e-4, atol=1e-5))
    if not (zi_ok and idx_ok and shapes_ok and uniform_ok and dist_ok):
        return _np_reference(n_diff, n_dist, atom_i_idx, j_elems, eta2, R_s,
                             R_c2, zeta, Lambda, eta4, R_c4, n_atoms, n_nei)

    xyz16 = np.ascontiguousarray(
        n_diff.reshape(A_TOT, N_NEI, 3).transpose(0, 2, 1).astype(np.float16)
    ).reshape(A_TOT, 3 * N_NEI)
    jr = j_elems.reshape(A_TOT, N_NEI)
    je1 = np.packbits(jr == 1, axis=1, bitorder="little")
    je8 = np.packbits(jr == 8, axis=1, bitorder="little")

    key = (eta2, R_s, R_c2, zeta, Lambda, eta4, R_c4)
    try:
        host = _get_exec(key).run([xyz16, je1, je8])
    except Exception:
        # transient tunnel/device failure: rebuild the executor once,
        # then fall back to the (slow but exact) numpy path
        try:
            _EXEC.pop(tuple(np.asarray(a, np.float64).tobytes()
                            for a in key), None)
            host = _get_exec(key).run([xyz16, je1, je8])
        except Exception:
            return _np_reference(n_diff, n_dist, atom_i_idx, j_elems, eta2,
                                 R_s, R_c2, zeta, Lambda, eta4, R_c4,
                                 n_atoms, n_nei)
    # device emits f16 to halve the downlink; the contract is f32
    return host[0].astype(np.float32).reshape(A_TOT, 4 * F)


# revision 43
# speedup vs baseline: 1.0091x; 1.0091x over previous
"""Behler-Parrinello symmetry-function fingerprints on 8 Trainium2 NeuronCores.

Layout: data-parallel over atoms (1024 atoms/core), partition = atom,
per-atom N*N neighbor-pair work in the free dimension.

Host/tunnel optimizations (the dispatch path dominates wall-clock here):
  - one cached jax.jit(shard_map(bass_exec)) executor per process — the
    stock run_bass_kernel_spmd re-traces and re-lowers on every call;
  - minimal payload: n_diff ships as float16 [A, 3N] (transposed on host)
    and the j_elems==1 / ==8 masks as packbits bit-planes (3 B/atom each);
    n_dist, 1/d and the masks are derived on-device, so ~1.2 MB goes up
    instead of 4.6 MB, and the f16 output halves the downlink;
  - the donated output buffers ping-pong: the kernel writes every output
    element, so call N donates call N-1's device-resident output instead
    of uploading fresh zero buffers;
  - whole-core batching: the per-neighbor tables run as [128, 192] ops
    (all 8 atom-tiles at once) with one DMA per tensor instead of 24.

Math restructurings vs the reference:
  - cos_jk = u_j . u_k from unit vectors; d_jk via law of cosines
    (sq = dj^2 + dk^2 - 2 dj dk cos), clamped to [0, Rc] so that the
    (1 + cos(pi d/Rc)) factor vanishes at/beyond the cutoff (mask-free).
  - exp(-eta4 (rj^2+rk^2)) * fc(rj) fc(rk) is separable: folded into
    per-neighbor tables h[j], h[k] together with the element masks.
  - (1 +/- cos)^zeta via repeated squaring (zeta = 1,2,4,16).
  - per-feature fused multiply+reduce (scalar_tensor_tensor accum_out)
    with the 0.125 * 2^(1-zeta) constant baked into the scalar operand.
  - g4_11 upper triangle = 0.5 * (full sum - diagonal); diagonal has
    cos = 1, d_jj = 0 so it reduces to an analytic per-neighbor sum.
"""
import numpy as np

A_TOT = 8192
N_NEI = 24
F = 8
N_CORES = 8
A_CORE = A_TOT // N_CORES      # 1024
P = 128                        # partitions (atoms per tile)
NTILES = A_CORE // P           # 8

_BUILT = {}
_EXEC = {}


def _np_reference(n_diff, n_dist, atom_i_idx, j_elems, eta2, R_s, R_c2,
                  zeta, Lambda, eta4, R_c4, n_atoms, n_nei):
    """Pure-numpy fallback (exact reference semantics), chunked over atoms."""
    dt = np.float32
    m1 = (j_elems == 1).astype(dt)
    m8 = (j_elems == 8).astype(dt)

    def fc(d, R_c):
        return 0.5 * (np.cos(np.pi * d / R_c) + 1.0)

    d = n_dist[:, None]
    out_g2 = []
    for m in (m1, m8):
        sf = np.exp(-eta2 * (d - R_s) ** 2) * fc(d, R_c2) * m[:, None]
        acc = np.zeros((n_atoms, F), dt)
        np.add.at(acc, atom_i_idx, sf)
        out_g2.append(acc)

    diff = n_diff.reshape(n_atoms, n_nei, 3)
    dist = n_dist.reshape(n_atoms, n_nei)
    jm1 = m1.reshape(n_atoms, n_nei)
    jm8 = m8.reshape(n_atoms, n_nei)

    def g4(jm, km, same):
        res = np.zeros((n_atoms, F), dt)
        CH = 256
        for s in range(0, n_atoms, CH):
            e = min(s + CH, n_atoms)
            dj = diff[s:e] * jm[s:e][..., None]
            dk = diff[s:e] * km[s:e][..., None]
            rj = dist[s:e] * jm[s:e]
            rk = dist[s:e] * km[s:e]
            dot = np.einsum('anc,amc->anm', dj, dk)
            rp = rj[:, :, None] * rk[:, None, :]
            valid = rp > 0
            if same:
                valid = valid & np.triu(np.ones((n_nei, n_nei), bool), k=1)
            cos = dot / np.where(valid, rp, 1.0)
            sq = ((dk[:, None, :, :] - dj[:, :, None, :]) ** 2).sum(-1)
            djk = np.sqrt(np.where(sq > 0, sq, 1.0))
            djk = np.where(sq > 0, djk, 0.0)
            valid = valid & (djk < R_c4[0])
            p1 = (cos[..., None] * Lambda + 1.0) ** zeta
            p2 = np.exp(-eta4 * (rj[:, :, None] ** 2
                                 + rk[:, None, :] ** 2)[..., None])
            p3 = (fc(rj[:, :, None, None], R_c4) * fc(rk[:, None, :, None],
                                                      R_c4)
                  * fc(djk[..., None], R_c4))
            term = p1 * p2 * p3 * (2.0 ** (1.0 - zeta)) * valid[..., None]
            res[s:e] = term.sum(axis=(1, 2))
        return res

    return np.concatenate([out_g2[0], out_g2[1],
                           g4(jm1, jm8, False), g4(jm1, jm1, True)], axis=1)


# Engine assignment knobs (tuned against the cost-model timeline sim):
#   n_stt_gp: how many of the 16 fused accumulate ops run on GPSIMD
#   sq_plan:  engine per squaring op in chain order ("a"=ACT, "v"=DVE, "g"=GP)
# NOTE: gpsimd.scalar_tensor_tensor does not compile on this toolchain
# (walrus lower_dve rejects it) -> all fused accumulates stay on DVE and
# GPSIMD gets plain tensor_tensor / tensor_scalar work instead.
PLAN = {
    "n_stt_gp": 0,
    "sq_plan": "aaaaaaaa",
    "cc_add_gp": True,
    "gw8_gp": True,
    "t1_gp": True,
    "gh1_split_gp": True,
}


def _build_nc(eta2, R_s, R_c2, zeta, Lambda, eta4u, R_c4u, ntiles=NTILES,
              plan=None):
    """Build the per-core Bass program. All hyper-params baked as constants.

    Inputs: xyz [A_CORE, 3N] f16 in (c n) order; je1/je8 [A_CORE, 3] u8
    packbits bit-planes of j_elems==1 / ==8. n_dist, 1/d and the masks
    are derived on-device.
    """
    import concourse.bass as bass
    import concourse.tile as tile
    from concourse import bacc, mybir

    if plan is None:
        plan = PLAN
    f32 = mybir.dt.float32
    f16 = mybir.dt.float16
    u8 = mybir.dt.uint8
    Alu = mybir.AluOpType
    Act = mybir.ActivationFunctionType
    N = N_NEI
    rs_zero = bool(np.all(R_s == 0.0))
    rc2_shared = bool(np.all(R_c2 == R_c2[0]))
    rc2u = float(R_c2[0])
    zi = [int(z) for z in zeta]
    assert all(abs(z - iz) < 1e-6 and iz >= 1 for z, iz in zip(zeta, zi))
    # per-feature constant: 2^(1-zeta)/8 (0.125 from the three 0.5 fc factors)
    sc = [0.125 * (2.0 ** (1.0 - z)) for z in zeta]

    nc = bacc.Bacc("TRN2", target_bir_lowering=False, debug=False)
    xyz_in = nc.dram_tensor("xyz", [A_CORE, 3 * N], f16, kind="ExternalInput")
    # two bit-planes (j_elems==1, j_elems==8), 3 bytes each per atom
    NB = N // 8
    je1_in = nc.dram_tensor("je1", [A_CORE, NB], u8, kind="ExternalInput")
    je8_in = nc.dram_tensor("je8", [A_CORE, NB], u8, kind="ExternalInput")
    out_dr = nc.dram_tensor("out", [A_CORE, 4 * F], f16, kind="ExternalOutput")

    T = ntiles
    TN = T * N

    with tile.TileContext(nc) as tc:
        with (
            tc.tile_pool(name="singles", bufs=1) as singles,
            tc.tile_pool(name="core", bufs=1) as corep,
            tc.tile_pool(name="big", bufs=3) as big,
        ):
            half_pi = singles.tile([P, 1], f32)
            nc.vector.memset(half_pi[:], float(np.pi / 2))
            ln_half = singles.tile([P, 1], f32)
            nc.vector.memset(ln_half[:], float(np.log(0.5)))
            halfc = singles.tile([P, 1], f32)
            nc.vector.memset(halfc[:], 0.5)
            rc_c = singles.tile([P, 1], f32)
            nc.vector.memset(rc_c[:], float(R_c4u))
            mhalf_pi = singles.tile([P, 1], f32)
            nc.vector.memset(mhalf_pi[:], float(-np.pi / 2))
            bitpos = singles.tile([P, 8], u8)
            for k in range(8):
                nc.vector.memset(bitpos[:, k:k + 1], 1 << k)

            def t3(ap2d):
                # [P, TN] contiguous -> [P, T, N] view
                return ap2d.rearrange("p (t n) -> p t n", n=N)

            # ---- whole-core loads: one DMA per input ------------------
            u16a = corep.tile([P, T, 3, N], f16)
            nc.sync.dma_start(u16a[:], xyz_in.rearrange(
                "(t p) (c n) -> p t c n", t=T, c=3))
            je1a = corep.tile([P, T, NB], u8)
            nc.sync.dma_start(je1a[:], je1_in.rearrange(
                "(t p) b -> p t b", t=T))
            je8a = corep.tile([P, T, NB], u8)
            nc.sync.dma_start(je8a[:], je8_in.rearrange(
                "(t p) b -> p t b", t=T))

            # ---- whole-core tables, [P, TN] flat ----------------------
            sq3a = corep.tile([P, T, 3, N], f32)
            nc.scalar.square(sq3a[:], u16a[:])
            dsqa = corep.tile([P, TN], f32)
            nc.vector.tensor_add(t3(dsqa[:]), sq3a[:, :, 0, :],
                                 sq3a[:, :, 1, :])
            nc.gpsimd.tensor_add(t3(dsqa[:]), t3(dsqa[:]), sq3a[:, :, 2, :])
            da = corep.tile([P, TN], f32)
            nc.scalar.sqrt(da[:], dsqa[:])
            rinva = corep.tile([P, TN], f32)
            nc.vector.reciprocal(rinva[:], da[:])
            ua = corep.tile([P, 3, TN], f32)
            for c in range(3):
                nc.vector.tensor_mul(t3(ua[:, c, :]), u16a[:, :, c, :],
                                     t3(rinva[:]))

            bp_b = bitpos[:].unsqueeze(1).broadcast_to([P, T * NB, 8])
            mb1 = corep.tile([P, T * NB, 8], u8)
            nc.vector.tensor_tensor(
                mb1[:], je1a[:].rearrange("p t b -> p (t b)")
                .unsqueeze(2).broadcast_to([P, T * NB, 8]),
                bp_b, Alu.bitwise_and)
            mb8 = corep.tile([P, T * NB, 8], u8)
            nc.vector.tensor_tensor(
                mb8[:], je8a[:].rearrange("p t b -> p (t b)")
                .unsqueeze(2).broadcast_to([P, T * NB, 8]),
                bp_b, Alu.bitwise_and)
            mf1 = corep.tile([P, TN], f32)
            nc.gpsimd.tensor_scalar_add(
                mf1[:], mb1[:].rearrange("p a b -> p (a b)"), 0.0)
            mf8 = corep.tile([P, TN], f32)
            nc.gpsimd.tensor_scalar_add(
                mf8[:], mb8[:].rearrange("p a b -> p (a b)"), 0.0)
            m1a = corep.tile([P, TN], f32)
            nc.vector.tensor_scalar(m1a[:], mf1[:], 0.0, None, Alu.is_gt)
            m8a = corep.tile([P, TN], f32)
            nc.vector.tensor_scalar(m8a[:], mf8[:], 0.0, None, Alu.is_gt)

            # min(d, Rc) = Rc - Relu(Rc - d); fold into the Sin bias:
            # sin(-pi/Rc*min + pi/2) = sin(pi/Rc*Relu(Rc-d) - pi/2)
            dcl = corep.tile([P, TN], f32)
            nc.scalar.activation(dcl[:], da[:], Act.Relu, bias=rc_c[:],
                                 scale=-1.0)
            q24a = corep.tile([P, TN], f32)
            nc.scalar.activation(q24a[:], dcl[:], Act.Sin,
                                 bias=mhalf_pi[:],
                                 scale=float(np.pi / R_c4u))
            e4ta = corep.tile([P, TN], f32)
            nc.scalar.activation(e4ta[:], dsqa[:], Act.Exp,
                                 scale=float(-eta4u))
            basea = corep.tile([P, TN], f32)
            nc.vector.scalar_tensor_tensor(basea[:], q24a[:], 1.0, e4ta[:],
                                           op0=Alu.add, op1=Alu.mult)
            h1a = corep.tile([P, TN], f32)
            nc.vector.tensor_mul(h1a[:], basea[:], m1a[:])
            h8a = corep.tile([P, TN], f32)
            nc.vector.tensor_mul(h8a[:], basea[:], m8a[:])
            hsqa = corep.tile([P, TN], f32)
            nc.gpsimd.tensor_mul(hsqa[:], h1a[:], h1a[:])
            hsa = corep.tile([P, T], f32)
            nc.vector.reduce_sum(hsa[:], t3(hsqa[:]),
                                 axis=mybir.AxisListType.X)
            # f16 copies of the h tables for the 2x-rate DVE pair stage
            h1h = corep.tile([P, TN], f16)
            nc.scalar.copy(h1h[:], h1a[:])
            h8h = corep.tile([P, TN], f16)
            nc.scalar.copy(h8h[:], h8a[:])

            # ---- G2 ---------------------------------------------------
            if rc2_shared and abs(rc2u - R_c4u) < 1e-12:
                q22a = q24a
            else:
                q22a = corep.tile([P, TN], f32)
                dc2 = corep.tile([P, TN], f32)
                nc.gpsimd.tensor_scalar(dc2[:], da[:], 0.0, rc2u,
                                        Alu.max, Alu.min)
                nc.scalar.activation(q22a[:], dc2[:], Act.Sin,
                                     bias=half_pi[:],
                                     scale=float(-np.pi / rc2u))
            hg1a = corep.tile([P, TN], f32)
            nc.vector.scalar_tensor_tensor(hg1a[:], q22a[:], 1.0, m1a[:],
                                           op0=Alu.add, op1=Alu.mult)
            hg8a = corep.tile([P, TN], f32)
            nc.vector.scalar_tensor_tensor(hg8a[:], q22a[:], 1.0, m8a[:],
                                           op0=Alu.add, op1=Alu.mult)
            e2ba = corep.tile([P, F, TN], f32)
            for f in range(F):
                if rs_zero:
                    nc.scalar.activation(e2ba[:, f, :], dsqa[:], Act.Exp,
                                         bias=ln_half[:],
                                         scale=float(-eta2[f]))
                else:
                    dsh = corep.tile([P, TN], f32, tag=f"dsh{f}")
                    nc.gpsimd.tensor_scalar_sub(dsh[:], da[:],
                                                float(R_s[f]))
                    dshs = corep.tile([P, TN], f32, tag=f"dshs{f}")
                    nc.scalar.square(dshs[:], dsh[:])
                    nc.scalar.activation(e2ba[:, f, :], dshs[:], Act.Exp,
                                         bias=ln_half[:],
                                         scale=float(-eta2[f]))
            outa = corep.tile([P, T, 4 * F], f32)
            g2pa = corep.tile([P, F, TN], f32)
            nc.vector.tensor_mul(
                g2pa[:], e2ba[:],
                hg1a[:].unsqueeze(1).broadcast_to([P, F, TN]))
            g2acc1 = corep.tile([P, F, T], f32)
            for f in range(F):
                nc.vector.reduce_sum(g2acc1[:, f, :], t3(g2pa[:, f, :]),
                                     axis=mybir.AxisListType.X)
            g2pa8 = corep.tile([P, F, TN], f32)
            nc.vector.tensor_mul(
                g2pa8[:], e2ba[:],
                hg8a[:].unsqueeze(1).broadcast_to([P, F, TN]))
            g2acc8 = corep.tile([P, F, T], f32)
            for f in range(F):
                nc.vector.reduce_sum(g2acc8[:, f, :], t3(g2pa8[:, f, :]),
                                     axis=mybir.AxisListType.X)
            nc.vector.tensor_copy(outa[:, :, 0:F],
                                  g2acc1[:].rearrange("p f t -> p t f"))
            nc.vector.tensor_copy(outa[:, :, F:2 * F],
                                  g2acc8[:].rearrange("p f t -> p t f"))

                # ---- G4 pair stage -------------------------------------
            # ---- G4 pair stage, per tile of 128 atoms -----------------
            t11va = corep.tile([P, T, F], f32)

            def emit_pairs(it):
                tsl = slice(it * N, (it + 1) * N)
                d_t = da[:, tsl]
                dsq_t = dsqa[:, tsl]
                h1_t = h1a[:, tsl]
                h1h_t = h1h[:, tsl]
                h8h_t = h8h[:, tsl]

                def jb(sl):   # value varies with j, broadcast along k
                    return sl.unsqueeze(2).broadcast_to([P, N, N])

                def kb(sl):   # value varies with k, broadcast along j
                    return sl.unsqueeze(1).broadcast_to([P, N, N])

                CC = big.tile([P, N, N], f32, tag="CC")
                tmp1 = big.tile([P, N, N], f32, tag="tmp1")
                tmp2 = big.tile([P, N, N], f32, tag="tmp2")
                ux = ua[:, 0, tsl]
                uy = ua[:, 1, tsl]
                uz = ua[:, 2, tsl]
                cc_add_eng = nc.gpsimd if plan["cc_add_gp"] else nc.vector
                nc.gpsimd.tensor_mul(CC[:], jb(ux), kb(ux))
                nc.gpsimd.tensor_mul(tmp1[:], jb(uy), kb(uy))
                nc.vector.tensor_mul(tmp2[:], jb(uz), kb(uz))
                cc_add_eng.tensor_add(CC[:], CC[:], tmp1[:])
                cc_add_eng.tensor_add(CC[:], CC[:], tmp2[:])

                S = big.tile([P, N, N], f32, tag="S")
                nc.gpsimd.tensor_add(S[:], jb(dsq_t), kb(dsq_t))
                RP = big.tile([P, N, N], f32, tag="RP")
                nc.gpsimd.tensor_mul(RP[:], jb(d_t), kb(d_t))
                T1 = big.tile([P, N, N], f32, tag="T1")
                (nc.gpsimd if plan["t1_gp"] else nc.vector).tensor_mul(
                    T1[:], RP[:], CC[:])
                SQ = big.tile([P, N, N], f32, tag="SQ")
                nc.vector.scalar_tensor_tensor(SQ[:], T1[:], -2.0, S[:],
                                               op0=Alu.mult, op1=Alu.add)
                SQc = big.tile([P, N, N], f32, tag="SQc")
                nc.scalar.activation(SQc[:], SQ[:], Act.Relu)
                DJK = big.tile([P, N, N], f32, tag="DJK")
                nc.scalar.sqrt(DJK[:], SQc[:])
                DJKc = big.tile([P, N, N], f32, tag="DJKc")
                nc.scalar.activation(DJKc[:], DJK[:], Act.Relu,
                                     bias=rc_c[:], scale=-1.0)
                Q4 = big.tile([P, N, N], f32, tag="Q4")
                nc.scalar.activation(Q4[:], DJKc[:], Act.Sin,
                                     bias=mhalf_pi[:],
                                     scale=float(np.pi / R_c4u))
                # f16 from here down the multiply+reduce path: TRN2's DVE
                # runs 16-bit element ops at 2x rate, and the rescaled
                # bases (1 +/- cos)/2 live in [0,1] so f16 cannot overflow
                GH1 = big.tile([P, N, N], f16, tag="GH1")
                if plan["gh1_split_gp"]:
                    # (1 + Q4) on ACT (it has the most slack), mul on GP;
                    # reuse the tmp1 tag's buffer ring (SBUF is tight)
                    A4 = big.tile([P, N, N], f32, tag="tmp1")
                    nc.scalar.activation(A4[:], Q4[:], Act.Identity,
                                         bias=1.0)
                    nc.gpsimd.tensor_mul(GH1[:], A4[:], jb(h1_t))
                else:
                    nc.vector.scalar_tensor_tensor(GH1[:], Q4[:], 1.0,
                                                   jb(h1_t), op0=Alu.add,
                                                   op1=Alu.mult)
                GW8 = big.tile([P, N, N], f16, tag="GW8")
                (nc.gpsimd if plan["gw8_gp"] else nc.vector).tensor_mul(
                    GW8[:], GH1[:], kb(h8h_t))
                GW1 = big.tile([P, N, N], f16, tag="GW1")
                nc.gpsimd.tensor_mul(GW1[:], GH1[:], kb(h1h_t))

                # powers (1 +/- CC)^z via squaring chains
                need_p = sorted({zi[f] for f in range(F) if Lambda[f] > 0})
                need_m = sorted({zi[f] for f in range(F) if Lambda[f] < 0})
                pows = {}
                sq_ct = [0]

                def mk_sq(dst, src):
                    c = plan["sq_plan"][sq_ct[0] % len(plan["sq_plan"])]
                    if c == "a":
                        nc.scalar.square(dst[:], src[:])
                    elif c == "g":
                        nc.gpsimd.tensor_mul(dst[:], src[:], src[:])
                    else:
                        nc.vector.tensor_mul(dst[:], src[:], src[:])
                    sq_ct[0] += 1

                for sign, need in (("p", need_p), ("m", need_m)):
                    if not need:
                        continue
                    b1 = big.tile([P, N, N], f16, tag=f"pow{sign}1")
                    if sign == "p":
                        nc.scalar.activation(b1[:], CC[:], Act.Identity,
                                             scale=0.5, bias=halfc[:])
                    else:
                        nc.scalar.activation(b1[:], CC[:], Act.Copy,
                                             scale=-0.5, bias=0.5)
                    pows[(sign, 1)] = b1
                    maxz = max(need)
                    z = 1
                    while z < maxz:
                        src = pows[(sign, z)]
                        z *= 2
                        dst = big.tile([P, N, N], f16, tag=f"pow{sign}{z}")
                        mk_sq(dst, src)
                        pows[(sign, z)] = dst
                    for z in need:
                        if (sign, z) in pows:
                            continue
                        acc = None
                        bit = 1
                        rem = z
                        while rem:
                            if rem & 1:
                                term = pows[(sign, bit)]
                                if acc is None:
                                    acc = term
                                else:
                                    na = big.tile([P, N, N], f16,
                                                  tag=f"pw{sign}{z}a{bit}")
                                    nc.vector.tensor_mul(na[:], acc[:],
                                                         term[:])
                                    acc = na
                            rem >>= 1
                            bit *= 2
                        pows[(sign, z)] = acc

                # fused per-feature multiply+reduce; split across DVE/GPSIMD.
                # Each engine accumulates into its own tiles to avoid
                # cross-engine false deps on a shared output tile.
                scratch = big.tile([P, N, N], f16, tag="scratch")
                scratch_g = big.tile([P, N, N], f16, tag="scratch_g")
                n_gp = plan["n_stt_gp"]
                stt_i = [0]

                def acc_stt(Pf, scale, GWv, GWg, accv, accg):
                    # distribute the 16 accumulate ops over DVE and GPSIMD
                    i = stt_i[0] % 16
                    use_gp = ((i + 1) * n_gp) // 16 > (i * n_gp) // 16
                    if use_gp:
                        nc.gpsimd.scalar_tensor_tensor(
                            scratch_g[:], Pf[:], float(scale), GWg[:],
                            op0=Alu.mult, op1=Alu.mult, accum_out=accg)
                    else:
                        nc.vector.scalar_tensor_tensor(
                            scratch[:], Pf[:], float(scale), GWv[:],
                            op0=Alu.mult, op1=Alu.mult, accum_out=accv)
                    stt_i[0] += 1
                    return use_gp

                for f in range(F):
                    sign = "p" if Lambda[f] > 0 else "m"
                    Pf = pows[(sign, zi[f])]
                    oc = outa[:, it, 2 * F + f:2 * F + f + 1]
                    acc_stt(Pf, 0.25, GW8, GW8, oc, oc)
                    if Lambda[f] > 0:
                        acc11 = t11va[:, it, f:f + 1]
                    else:
                        acc11 = outa[:, it, 3 * F + f:3 * F + f + 1]
                    acc_stt(Pf, 0.125, GW1, GW1, acc11, acc11)

            for it in range(ntiles):
                emit_pairs(it)

            # diagonal fix for Lambda=+1 features, batched over tiles
            for f in range(F):
                if Lambda[f] > 0:
                    kap = 0.25
                    nc.vector.scalar_tensor_tensor(
                        outa[:, :, 3 * F + f], hsa[:], float(-kap),
                        t11va[:, :, f], op0=Alu.mult, op1=Alu.add)

            out16a = corep.tile([P, T, 4 * F], f16)
            nc.scalar.copy(out16a[:], outa[:])
            nc.sync.dma_start(out_dr.rearrange("(t p) f -> p t f", t=T),
                              out16a[:])

    nc.compile()
    return nc


def _get_nc(key_arrays):
    key = tuple(np.asarray(a, np.float64).tobytes() for a in key_arrays)
    if key not in _BUILT:
        eta2, R_s, R_c2, zeta, Lambda, eta4, R_c4 = key_arrays
        _BUILT[key] = _build_nc(eta2, R_s, R_c2, zeta, Lambda,
                                float(eta4[0]), float(R_c4[0]))
    return _BUILT[key]


class _CachedExec:
    """One persistent jitted shard_map executor around a Bass program.

    Re-tracing/compiling per call (what run_bass_kernel_spmd does under
    axon) costs ~150 ms; this pays it once. The donated output operand
    ping-pongs: the program writes every output element, so call N hands
    back call N-1's device-resident output instead of uploading zeros.
    """

    def __init__(self, nc):
        import jax
        from jax.sharding import Mesh, PartitionSpec
        from jax.experimental.shard_map import shard_map
        from concourse import mybir
        from concourse.bass2jax import (_bass_exec_p, install_neuronx_cc_hook,
                                        partition_id_tensor)

        install_neuronx_cc_hook()
        partition_name = (nc.partition_id_tensor.name
                          if nc.partition_id_tensor else None)
        in_names, out_names, out_avals, zero_outs = [], [], [], []
        for alloc in nc.m.functions[0].allocations:
            if not isinstance(alloc, mybir.MemoryLocationSet):
                continue
            name = alloc.memorylocations[0].name
            if alloc.kind == "ExternalInput":
                if name != partition_name:
                    in_names.append(name)
            elif alloc.kind == "ExternalOutput":
                out_names.append(name)
                shape = tuple(alloc.tensor_shape)
                dtype = mybir.dt.np(alloc.dtype)
                out_avals.append(jax.core.ShapedArray(shape, dtype))
                zero_outs.append(
                    np.zeros((N_CORES * shape[0], *shape[1:]), dtype))
        n_params = len(in_names)
        all_in = list(in_names) + list(out_names)
        if partition_name is not None:
            all_in.append(partition_name)

        def _body(*args):
            operands = list(args)
            if partition_name is not None:
                operands.append(partition_id_tensor())
            return tuple(_bass_exec_p.bind(
                *operands,
                out_avals=tuple(out_avals),
                in_names=tuple(all_in),
                out_names=tuple(out_names),
                lowering_input_output_aliases=(),
                sim_require_finite=True,
                sim_require_nnan=True,
                nc=nc,
            ))

        devices = jax.devices()[:N_CORES]
        mesh = Mesh(np.asarray(devices), ("core",))
        spec = (PartitionSpec("core"),)
        n_out = len(out_names)
        self._fn = jax.jit(
            shard_map(_body, mesh=mesh,
                      in_specs=spec * (n_params + n_out),
                      out_specs=spec * n_out, check_rep=False),
            donate_argnums=tuple(range(n_params, n_params + n_out)),
            keep_unused=True)
        self._in_names = in_names
        self._zero_outs = zero_outs
        self._donate = None

    def run(self, arrs):
        donate = self._donate if self._donate is not None else self._zero_outs
        self._donate = None
        outs = self._fn(*arrs, *donate)
        host = [np.asarray(o) for o in outs]
        self._donate = list(outs)
        return host


def _get_exec(key_arrays):
    key = tuple(np.asarray(a, np.float64).tobytes() for a in key_arrays)
    if key not in _EXEC:
        _EXEC[key] = _CachedExec(_get_nc(key_arrays))
    return _EXEC[key]


def kernel(n_diff, n_dist, atom_i_idx, j_elems, eta2, R_s, R_c2,
           zeta, Lambda, eta4, R_c4, n_atoms, n_nei):
    n_diff = np.asarray(n_diff, np.float32)
    n_dist = np.asarray(n_dist, np.float32)
    atom_i_idx = np.asarray(atom_i_idx)
    j_elems = np.asarray(j_elems)
    eta2 = np.asarray(eta2, np.float32)
    R_s = np.asarray(R_s, np.float32)
    R_c2 = np.asarray(R_c2, np.float32)
    zeta = np.asarray(zeta, np.float32)
    Lambda = np.asarray(Lambda, np.float32)
    eta4 = np.asarray(eta4, np.float32)
    R_c4 = np.asarray(R_c4, np.float32)
    n_atoms = int(n_atoms)
    n_nei = int(n_nei)

    zi_ok = bool(np.allclose(zeta, np.round(zeta)) and np.all(zeta >= 1))
    shapes_ok = (n_atoms == A_TOT and n_nei == N_NEI and len(eta2) == F)
    uniform_ok = bool(np.all(eta4 == eta4[0]) and np.all(R_c4 == R_c4[0])
                      and np.all(R_c2 == R_c2[0]))
    # Subsampled structural checks (a full scan costs ~5 ms of the
    # ~90 ms call): atom_i_idx must be the dense ragged pattern and
    # n_dist must equal |n_diff| (the device recomputes it from n_diff).
    ss = np.arange(0, n_atoms * n_nei, 617)
    idx_ok = (atom_i_idx.shape == (n_atoms * n_nei,)
              and bool(np.array_equal(atom_i_idx[ss], ss // n_nei)))
    nd_ss = n_diff.reshape(-1, 3)[ss]
    dist_ok = bool(np.allclose(n_dist[ss],
                               np.sqrt((nd_ss * nd_ss).sum(axis=1)),
                               rtol=1e-4, atol=1e-5))
    if not (zi_ok and idx_ok and shapes_ok and uniform_ok and dist_ok):
        return _np_reference(n_diff, n_dist, atom_i_idx, j_elems, eta2, R_s,
                             R_c2, zeta, Lambda, eta4, R_c4, n_atoms, n_nei)

    xyz16 = np.ascontiguousarray(
        n_diff.reshape(A_TOT, N_NEI, 3).transpose(0, 2, 1).astype(np.float16)
    ).reshape(A_TOT, 3 * N_NEI)
    jr = j_elems.reshape(A_TOT, N_NEI)
    je1 = np.packbits(jr == 1, axis=1, bitorder="little")
    je8 = np.packbits(jr == 8, axis=1, bitorder="little")

    key = (eta2, R_s, R_c2, zeta, Lambda, eta4, R_c4)
    try:
        host = _get_exec(key).run([xyz16, je1, je8])
    except Exception:
        # transient tunnel/device failure: rebuild the executor once,
        # then fall back to the (slow but exact) numpy path
        try:
            _EXEC.pop(tuple(np.asarray(a, np.float64).tobytes()
                            for a in key), None)
            host = _get_exec(key).run([xyz16, je1, je8])
        except Exception:
            return _np_reference(n_diff, n_dist, atom_i_idx, j_elems, eta2,
                                 R_s, R_c2, zeta, Lambda, eta4, R_c4,
                                 n_atoms, n_nei)
    # device emits f16 to halve the downlink; the contract is f32
    return host[0].astype(np.float32).reshape(A_TOT, 4 * F)
